# revision 7
# baseline (speedup 1.0000x reference)
"""V4: exact ragged gather + length-balanced core assignment — DMA only the rows [begin,end) actually needed.

Per core: 4 groups of 64 contiguous b's. Per group, the needed rows
(concat over b of seq[b, begin:end)) form a compacted stream, padded with
-1 indices to NT=128 tiles of 128 rows (NT covers the worst case 64*256).
dma_gather calls of 16 tiles each carry a runtime valid-count
(value_load from SBUF), so trailing -1s cost no HBM traffic.

Reduction: per row-tile, build a [128, 64] selection matrix on DVE in one
tensor_scalar op: sel[k, j] = (colidx[k] == j) * w[k], where colidx is the
b-slot of row k within the group (-1 for padding) and w = 1/len. Then
TensorE: psum[dc][:, 0:64] += gtile[K=128, M=128dc].T @ sel[K=128, N=64],
accumulated over all NT tiles of the group. Output [D, BL] d-major,
host transposes back.
"""

import numpy as np

import concourse.bass as bass
from concourse import bacc
import concourse.mybir as mybir
import concourse.tile as tile
from concourse.bass_utils import run_bass_kernel_spmd

B, L, D = 2048, 512, 512
NCORES = 8
BL = B // NCORES  # 256
GB = 64  # b's per group (region = GB*L = 32768 rows, int16 idx max)
NT = (GB * 256) // 128  # 128 row-tiles per group (worst case: all len=256)
CT = 8  # tiles per dma_gather call (8*128=1024 idx; >1024 wedges Q7)
GRPS = BL // GB  # 4 groups per core
CALLS_PER_GRP = NT // CT  # 8
NCALLS = GRPS * CALLS_PER_GRP  # 32

_CACHE = {}
LAST_RESULTS = None
LAST_SPMD = None
STATIC_CNTS = None  # tlsim-only: per-call static num_idxs specialization
RACE_CHECK = True


def _build_bass():
    nc = bacc.Bacc("TRN2", detect_race_conditions=RACE_CHECK)
    f32 = mybir.dt.float32
    i32 = mybir.dt.int32
    seq = nc.dram_tensor("seq", [BL, L, D], f32, kind="ExternalInput")
    # per-tile selection inputs: columns 2t = colidx, 2t+1 = w  (GRPS*NT tiles)
    colw = nc.dram_tensor("colw", [128, GRPS * NT * 2], f32, kind="ExternalInput")
    gidx = nc.dram_tensor("gidx", [128, NCALLS * CT * 128 // 16], mybir.dt.int16,
                          kind="ExternalInput")
    gcnt = nc.dram_tensor("gcnt", [1, NCALLS], i32, kind="ExternalInput")
    iotaf = nc.dram_tensor("iotaf", [128, GB], f32, kind="ExternalInput")
    outt = nc.dram_tensor("outt", [D, BL], f32, kind="ExternalOutput")

    rows = seq[:].rearrange("b l d -> (b l) d")  # [BL*L, D]
    idx_cols = CT * 128 // 16  # idx columns per call (128)

    with tile.TileContext(nc) as tc:
        with (
            tc.tile_pool(name="gpool", bufs=4) as gpool,
            tc.tile_pool(name="selp", bufs=6) as selp,
            tc.tile_pool(name="constp", bufs=1) as constp,
            tc.tile_pool(name="psump", bufs=2, space="PSUM") as psump,
            tc.tile_pool(name="outp", bufs=2) as outp,
        ):
            colw_sb = constp.tile([128, GRPS * NT * 2], f32)
            nc.sync.dma_start(out=colw_sb[:], in_=colw[:])
            idx_sb = constp.tile([128, NCALLS * idx_cols], mybir.dt.int16)
            nc.sync.dma_start(out=idx_sb[:], in_=gidx[:])
            cnt_sb = constp.tile([1, NCALLS], i32)
            nc.sync.dma_start(out=cnt_sb[:], in_=gcnt[:])

            iota_f = constp.tile([128, GB], f32)
            nc.sync.dma_start(out=iota_f[:], in_=iotaf[:])

            # zero the 3 physical gather slots once: stale SBUF at boot may
            # hold NaN bit patterns, and NaN * 0-selection would poison psum.
            # The 3 init tiles are live concurrently -> 3 distinct slots.
            ginit = [
                gpool.tile([128, CT * D], f32, tag="g", name=f"ginit{k}")
                for k in range(4)
            ]
            for tl in ginit:
                nc.any.memset(tl[:], 0.0)

            for grp in range(GRPS):
                psums = [
                    psump.tile([128, GB], f32, tag=f"ps{dc}", name=f"ps{dc}")
                    for dc in range(4)
                ]
                for call in range(CALLS_PER_GRP):
                    g = grp * CALLS_PER_GRP + call
                    gtile = gpool.tile([128, CT * D], f32, tag="g", name="gtile")
                    if STATIC_CNTS is None:
                        # no min/max: the runtime assert they emit wedges
                        # the device under this runtime (no notification path)
                        cnt_rv = nc.gpsimd.value_load(cnt_sb[0:1, g : g + 1])
                        nc.gpsimd.dma_gather(
                            gtile[:].rearrange("p (c e) -> p c e", e=D),
                            rows[grp * GB * L : (grp + 1) * GB * L, :],
                            idx_sb[:, g * idx_cols : (g + 1) * idx_cols],
                            CT * 128,
                            cnt_rv,
                            D,
                        )
                    else:
                        cnt = int(STATIC_CNTS[g])
                        ni = -(-cnt // 16) * 16  # round up to 16
                        nc.gpsimd.dma_gather(
                            gtile[:].rearrange("p (c e) -> p c e", e=D)[
                                :, : -(-ni // 128), :
                            ],
                            rows[grp * GB * L : (grp + 1) * GB * L, :],
                            idx_sb[:, g * idx_cols : g * idx_cols + ni // 16],
                            ni,
                            cnt,
                            D,
                        )
                    for t in range(CT):
                        tg = grp * NT + call * CT + t  # global tile id
                        sel = selp.tile([128, GB], f32, tag="sel", name="sel")
                        nc.vector.tensor_scalar(
                            out=sel[:],
                            in0=iota_f[:],
                            scalar1=colw_sb[:, 2 * tg : 2 * tg + 1],
                            scalar2=colw_sb[:, 2 * tg + 1 : 2 * tg + 2],
                            op0=mybir.AluOpType.is_equal,
                            op1=mybir.AluOpType.mult,
                        )
                        tile_first = call == 0 and t == 0
                        tile_last = call == CALLS_PER_GRP - 1 and t == CT - 1
                        for dc in range(4):
                            nc.tensor.matmul(
                                out=psums[dc][:],
                                lhsT=gtile[:, t * D + dc * 128 : t * D + (dc + 1) * 128],
                                rhs=sel[:],
                                start=tile_first,
                                stop=tile_last,
                            )
                out_sb = outp.tile([128, 4 * GB], f32, tag="out", name="out_sb")
                for dc in range(4):
                    nc.vector.tensor_copy(
                        out=out_sb[:, dc * GB : (dc + 1) * GB], in_=psums[dc][:]
                    )
                nc.sync.dma_start(
                    out=outt[:, grp * GB : (grp + 1) * GB].rearrange(
                        "(dc p) j -> p dc j", dc=4
                    ),
                    in_=out_sb[:].rearrange("p (dc j) -> p dc j", dc=4),
                )
    nc.compile()
    return nc


def _get_bass():
    if "nc" not in _CACHE:
        _CACHE["nc"] = _build_bass()
    return _CACHE["nc"]


def _host_prep(begin_c, end_c):
    """Compacted per-group gather indices, per-call counts, per-tile col/w."""
    length = (end_c - begin_c).astype(np.int64)
    w_b = 1.0 / length.astype(np.float32)
    idx_all = np.full((NCALLS * CT * 128,), -1, dtype=np.int64)
    colidx = np.full((GRPS * NT, 128), -1.0, dtype=np.float32)
    wcol = np.zeros((GRPS * NT, 128), dtype=np.float32)
    cnt = np.zeros(NCALLS, dtype=np.int32)
    for grp in range(GRPS):
        bs = np.arange(grp * GB, (grp + 1) * GB)
        lens = length[bs]
        n_rows = int(lens.sum())
        # stream of (slot, l) for all rows of the group, in slot order
        slots = np.repeat(np.arange(GB), lens)
        ls = np.concatenate([np.arange(begin_c[b], end_c[b]) for b in bs])
        ridx = slots * L + ls  # row index within group region
        base = grp * NT * 128
        idx_all[base : base + n_rows] = ridx
        tiles = np.arange(n_rows) // 128
        pos = np.arange(n_rows) % 128
        colidx[grp * NT + tiles, pos] = slots.astype(np.float32)
        wcol[grp * NT + tiles, pos] = w_b[bs][slots]
        for call in range(CALLS_PER_GRP):
            c = min(max(n_rows - call * CT * 128, 0), CT * 128)
            g = grp * CALLS_PER_GRP + call
            if c == 0:
                # avoid fully-empty calls (sim chokes; HW gains nothing)
                idx_all[g * CT * 128] = 0
                c = 1
            cnt[g] = c
    assert idx_all.max() < GB * L
    idx16 = idx_all.astype(np.int16).reshape(-1, 16).T  # [16, total/16]
    idx = np.ascontiguousarray(np.tile(idx16, (8, 1)))  # [128, total/16]
    # colw[p, 2t] = colidx, colw[p, 2t+1] = w
    colw = np.empty((128, GRPS * NT * 2), dtype=np.float32)
    colw[:, 0::2] = colidx.T
    colw[:, 1::2] = wcol.T
    cnt2 = cnt.reshape(1, NCALLS)
    return np.ascontiguousarray(colw), idx, np.ascontiguousarray(cnt2)


def _balanced_assignment(length):
    """Assign b's to cores, serpentine over descending length, so per-core
    total gathered rows (the DMA-bound cost) are near-equal."""
    order = np.argsort(-length, kind="stable")
    asm = np.empty((NCORES, BL), dtype=np.int64)
    for r in range(BL):
        cores = range(NCORES) if r % 2 == 0 else range(NCORES - 1, -1, -1)
        for j, c in enumerate(cores):
            asm[c, r] = order[r * NCORES + j]
    return asm


def kernel(seq, begin, end):
    global LAST_RESULTS, LAST_SPMD
    seq = np.ascontiguousarray(np.asarray(seq, dtype=np.float32))
    begin_i = np.asarray(begin).astype(np.int64)
    end_i = np.asarray(end).astype(np.int64)
    asm = _balanced_assignment(end_i - begin_i)

    nc = _get_bass()
    iota_np = np.broadcast_to(
        np.arange(GB, dtype=np.float32)[None, :], (128, GB)
    ).copy()
    in_maps = []
    for c in range(NCORES):
        bs = asm[c]
        colw, idx, cnt = _host_prep(begin_i[bs], end_i[bs])
        in_maps.append(
            {"seq": seq[bs], "colw": colw, "gidx": idx, "gcnt": cnt,
             "iotaf": iota_np}
        )

    LAST_SPMD = (nc, in_maps)
    LAST_RESULTS = run_bass_kernel_spmd(nc, in_maps, core_ids=list(range(NCORES)))
    out = np.empty((B, D), dtype=np.float32)
    for c in range(NCORES):
        out[asm[c]] = LAST_RESULTS.results[c]["outt"].T
    return out


# revision 8
# speedup vs baseline: 151.3876x; 151.3876x over previous
"""V4: exact ragged gather + length-balanced core assignment — DMA only the rows [begin,end) actually needed.

Per core: 4 groups of 64 contiguous b's. Per group, the needed rows
(concat over b of seq[b, begin:end)) form a compacted stream, padded with
-1 indices to NT=128 tiles of 128 rows (NT covers the worst case 64*256).
dma_gather calls of 16 tiles each carry a runtime valid-count
(value_load from SBUF), so trailing -1s cost no HBM traffic.

Reduction: per row-tile, build a [128, 64] selection matrix on DVE in one
tensor_scalar op: sel[k, j] = (colidx[k] == j) * w[k], where colidx is the
b-slot of row k within the group (-1 for padding) and w = 1/len. Then
TensorE: psum[dc][:, 0:64] += gtile[K=128, M=128dc].T @ sel[K=128, N=64],
accumulated over all NT tiles of the group. Output [D, BL] d-major,
host transposes back.
"""

import time

import numpy as np

import concourse.bass as bass
from concourse import bacc
import concourse.mybir as mybir
import concourse.tile as tile
from concourse.bass_utils import run_bass_kernel_spmd

B, L, D = 2048, 512, 512
NCORES = 8
BL = B // NCORES  # 256
GB = 64  # b's per group (region = GB*L = 32768 rows, int16 idx max)
NT = (GB * 256) // 128  # 128 row-tiles per group (worst case: all len=256)
CT = 8  # tiles per dma_gather call (8*128=1024 idx; >1024 wedges Q7)
GRPS = BL // GB  # 4 groups per core
CALLS_PER_GRP = NT // CT  # 8
NCALLS = GRPS * CALLS_PER_GRP  # 32

_CACHE = {}
LAST_RESULTS = None
LAST_SPMD = None
STATIC_CNTS = None  # tlsim-only: per-call static num_idxs specialization
RACE_CHECK = True


def _build_bass():
    nc = bacc.Bacc("TRN2", detect_race_conditions=RACE_CHECK)
    f32 = mybir.dt.float32
    i32 = mybir.dt.int32
    seq = nc.dram_tensor("seq", [BL, L, D], f32, kind="ExternalInput")
    # per-tile selection inputs: columns 2t = colidx, 2t+1 = w  (GRPS*NT tiles)
    colw = nc.dram_tensor("colw", [128, GRPS * NT * 2], f32, kind="ExternalInput")
    gidx = nc.dram_tensor("gidx", [128, NCALLS * CT * 128 // 16], mybir.dt.int16,
                          kind="ExternalInput")
    gcnt = nc.dram_tensor("gcnt", [1, NCALLS], i32, kind="ExternalInput")
    iotaf = nc.dram_tensor("iotaf", [128, GB], f32, kind="ExternalInput")
    outt = nc.dram_tensor("outt", [D, BL], f32, kind="ExternalOutput")

    rows = seq[:].rearrange("b l d -> (b l) d")  # [BL*L, D]
    idx_cols = CT * 128 // 16  # idx columns per call (128)

    with tile.TileContext(nc) as tc:
        with (
            tc.tile_pool(name="gpool", bufs=4) as gpool,
            tc.tile_pool(name="selp", bufs=6) as selp,
            tc.tile_pool(name="constp", bufs=1) as constp,
            tc.tile_pool(name="psump", bufs=2, space="PSUM") as psump,
            tc.tile_pool(name="outp", bufs=2) as outp,
        ):
            colw_sb = constp.tile([128, GRPS * NT * 2], f32)
            nc.sync.dma_start(out=colw_sb[:], in_=colw[:])
            idx_sb = constp.tile([128, NCALLS * idx_cols], mybir.dt.int16)
            nc.sync.dma_start(out=idx_sb[:], in_=gidx[:])
            cnt_sb = constp.tile([1, NCALLS], i32)
            nc.sync.dma_start(out=cnt_sb[:], in_=gcnt[:])

            iota_f = constp.tile([128, GB], f32)
            nc.sync.dma_start(out=iota_f[:], in_=iotaf[:])

            # zero the 3 physical gather slots once: stale SBUF at boot may
            # hold NaN bit patterns, and NaN * 0-selection would poison psum.
            # The 3 init tiles are live concurrently -> 3 distinct slots.
            ginit = [
                gpool.tile([128, CT * D], f32, tag="g", name=f"ginit{k}")
                for k in range(4)
            ]
            for tl in ginit:
                nc.any.memset(tl[:], 0.0)

            for grp in range(GRPS):
                psums = [
                    psump.tile([128, GB], f32, tag=f"ps{dc}", name=f"ps{dc}")
                    for dc in range(4)
                ]
                for call in range(CALLS_PER_GRP):
                    g = grp * CALLS_PER_GRP + call
                    gtile = gpool.tile([128, CT * D], f32, tag="g", name="gtile")
                    if STATIC_CNTS is None:
                        # no min/max: the runtime assert they emit wedges
                        # the device under this runtime (no notification path)
                        cnt_rv = nc.gpsimd.value_load(cnt_sb[0:1, g : g + 1])
                        nc.gpsimd.dma_gather(
                            gtile[:].rearrange("p (c e) -> p c e", e=D),
                            rows[grp * GB * L : (grp + 1) * GB * L, :],
                            idx_sb[:, g * idx_cols : (g + 1) * idx_cols],
                            CT * 128,
                            cnt_rv,
                            D,
                        )
                    else:
                        cnt = int(STATIC_CNTS[g])
                        ni = -(-cnt // 16) * 16  # round up to 16
                        nc.gpsimd.dma_gather(
                            gtile[:].rearrange("p (c e) -> p c e", e=D)[
                                :, : -(-ni // 128), :
                            ],
                            rows[grp * GB * L : (grp + 1) * GB * L, :],
                            idx_sb[:, g * idx_cols : g * idx_cols + ni // 16],
                            ni,
                            cnt,
                            D,
                        )
                    for t in range(CT):
                        tg = grp * NT + call * CT + t  # global tile id
                        sel = selp.tile([128, GB], f32, tag="sel", name="sel")
                        nc.vector.tensor_scalar(
                            out=sel[:],
                            in0=iota_f[:],
                            scalar1=colw_sb[:, 2 * tg : 2 * tg + 1],
                            scalar2=colw_sb[:, 2 * tg + 1 : 2 * tg + 2],
                            op0=mybir.AluOpType.is_equal,
                            op1=mybir.AluOpType.mult,
                        )
                        tile_first = call == 0 and t == 0
                        tile_last = call == CALLS_PER_GRP - 1 and t == CT - 1
                        for dc in range(4):
                            nc.tensor.matmul(
                                out=psums[dc][:],
                                lhsT=gtile[:, t * D + dc * 128 : t * D + (dc + 1) * 128],
                                rhs=sel[:],
                                start=tile_first,
                                stop=tile_last,
                            )
                out_sb = outp.tile([128, 4 * GB], f32, tag="out", name="out_sb")
                for dc in range(4):
                    nc.vector.tensor_copy(
                        out=out_sb[:, dc * GB : (dc + 1) * GB], in_=psums[dc][:]
                    )
                nc.sync.dma_start(
                    out=outt[:, grp * GB : (grp + 1) * GB].rearrange(
                        "(dc p) j -> p dc j", dc=4
                    ),
                    in_=out_sb[:].rearrange("p (dc j) -> p dc j", dc=4),
                )
    nc.compile()
    return nc


def _get_bass():
    if "nc" not in _CACHE:
        _CACHE["nc"] = _build_bass()
    return _CACHE["nc"]


def _host_prep(begin_c, end_c):
    """Compacted per-group gather indices, per-call counts, per-tile col/w."""
    length = (end_c - begin_c).astype(np.int64)
    w_b = 1.0 / length.astype(np.float32)
    idx_all = np.full((NCALLS * CT * 128,), -1, dtype=np.int64)
    colidx = np.full((GRPS * NT, 128), -1.0, dtype=np.float32)
    wcol = np.zeros((GRPS * NT, 128), dtype=np.float32)
    cnt = np.zeros(NCALLS, dtype=np.int32)
    for grp in range(GRPS):
        bs = np.arange(grp * GB, (grp + 1) * GB)
        lens = length[bs]
        n_rows = int(lens.sum())
        # stream of (slot, l) for all rows of the group, in slot order
        slots = np.repeat(np.arange(GB), lens)
        ls = np.concatenate([np.arange(begin_c[b], end_c[b]) for b in bs])
        ridx = slots * L + ls  # row index within group region
        base = grp * NT * 128
        idx_all[base : base + n_rows] = ridx
        tiles = np.arange(n_rows) // 128
        pos = np.arange(n_rows) % 128
        colidx[grp * NT + tiles, pos] = slots.astype(np.float32)
        wcol[grp * NT + tiles, pos] = w_b[bs][slots]
        for call in range(CALLS_PER_GRP):
            c = min(max(n_rows - call * CT * 128, 0), CT * 128)
            g = grp * CALLS_PER_GRP + call
            if c == 0:
                # avoid fully-empty calls (sim chokes; HW gains nothing)
                idx_all[g * CT * 128] = 0
                c = 1
            cnt[g] = c
    assert idx_all.max() < GB * L
    idx16 = idx_all.astype(np.int16).reshape(-1, 16).T  # [16, total/16]
    idx = np.ascontiguousarray(np.tile(idx16, (8, 1)))  # [128, total/16]
    # colw[p, 2t] = colidx, colw[p, 2t+1] = w
    colw = np.empty((128, GRPS * NT * 2), dtype=np.float32)
    colw[:, 0::2] = colidx.T
    colw[:, 1::2] = wcol.T
    cnt2 = cnt.reshape(1, NCALLS)
    return np.ascontiguousarray(colw), idx, np.ascontiguousarray(cnt2)


def _balanced_assignment(length):
    """Assign b's to cores, serpentine over descending length, so per-core
    total gathered rows (the DMA-bound cost) are near-equal."""
    order = np.argsort(-length, kind="stable")
    asm = np.empty((NCORES, BL), dtype=np.int64)
    for r in range(BL):
        cores = range(NCORES) if r % 2 == 0 else range(NCORES - 1, -1, -1)
        for j, c in enumerate(cores):
            asm[c, r] = order[r * NCORES + j]
    return asm


def kernel(seq, begin, end):
    global LAST_RESULTS, LAST_SPMD
    seq = np.ascontiguousarray(np.asarray(seq, dtype=np.float32))
    begin_i = np.asarray(begin).astype(np.int64)
    end_i = np.asarray(end).astype(np.int64)
    asm = _balanced_assignment(end_i - begin_i)

    nc = _get_bass()
    iota_np = np.broadcast_to(
        np.arange(GB, dtype=np.float32)[None, :], (128, GB)
    ).copy()
    in_maps = []
    for c in range(NCORES):
        bs = asm[c]
        colw, idx, cnt = _host_prep(begin_i[bs], end_i[bs])
        in_maps.append(
            {"seq": seq[bs], "colw": colw, "gidx": idx, "gcnt": cnt,
             "iotaf": iota_np}
        )

    LAST_SPMD = (nc, in_maps)
    # the axon-tunneled devices occasionally report a transient
    # NRT_EXEC_UNIT_UNRECOVERABLE; a fresh attempt recovers
    last_exc = None
    for attempt in range(3):
        try:
            LAST_RESULTS = run_bass_kernel_spmd(
                nc, in_maps, core_ids=list(range(NCORES))
            )
            break
        except Exception as e:  # noqa: BLE001
            last_exc = e
            time.sleep(10.0)
    else:
        raise last_exc
    out = np.empty((B, D), dtype=np.float32)
    for c in range(NCORES):
        out[asm[c]] = LAST_RESULTS.results[c]["outt"].T
    return out
